# revision 2
# baseline (speedup 1.0000x reference)
"""Trainium2 Bass kernel for nn_InverseHaarTransform.

out = sum_b upfirdn(band_b, f_b) for 4 bands; reformulated per (sample,ch) as
out = sum_g R_g @ u_pair_g @ C_g^T with banded 1024x512 operators derived from
rank-1 (SVD) factors of each 2x2 filter.

Implementation per NeuronCore (2 samples x 3 channels = 6 instances):
  - H-pass on TensorE: fp32 matmuls, stationary slab [K<=128, m=120] packs both
    bands of a column-factor group; rhs packs the two bands' input row windows.
  - W-pass on VectorE: scalar_tensor_tensor chains over column-shifted views of
    Y, writing even/odd output columns directly with stride-2 APs. Per-group
    pre-scales (folded into the ACT PSUM->SBUF copy) make chains end at coef 1.
  - Boundary output columns 0, 1, 1023 get exact small fixup ops.
Batch dim sharded 2-per-core across 8 cores; filters replicated.
"""
import sys
sys.path.insert(0, "/opt/trn_rl_repo")
import numpy as np
import concourse.bass as bass
import concourse.bacc as bacc
import concourse.tile as tile
import concourse.mybir as mybir
from concourse.bass_utils import run_bass_kernel_spmd

F32 = mybir.dt.float32
H = 512
OUT = 1024
BM = 120
N_CORES = 8
SPC = 2   # samples per core
CH = 3    # output channels


def _up_matrix(n):
    A = np.zeros((2 * n, n))
    A[0, 0] = 1.0
    for k in range(1, n):
        A[2 * k, k - 1] = 0.25
        A[2 * k, k] = 0.75
    for k in range(0, n - 1):
        A[2 * k + 1, k] = 0.75
        A[2 * k + 1, k + 1] = 0.25
    A[2 * n - 1, n - 1] = 1.0
    return A


def _band_matrix(rv, n):
    A = _up_matrix(n)
    S = np.zeros_like(A)
    S[1:] = A[:-1]
    return rv[0] * S + rv[1] * A


class _Group:
    def __init__(self, cv):
        self.cv = cv
        self.terms = []
        self.scale = 1.0

    @property
    def even_taps(self):
        c0, c1 = self.cv
        return (0.75 * c0 + 0.25 * c1, 0.25 * c0 + 0.75 * c1)

    @property
    def odd_taps(self):
        c0, c1 = self.cv
        return (0.25 * c0, 0.75 * (c0 + c1), 0.25 * c1)


def _decompose(filters):
    groups = []
    for b, f in enumerate(filters):
        U, s, Vt = np.linalg.svd(np.asarray(f, dtype=np.float64))
        for t in range(2):
            if s[t] <= 1e-7 * max(s[0], 1e-30):
                continue
            rv = U[:, t] * s[t]
            cv = Vt[t, :]
            j = int(np.argmax(np.abs(cv)))
            if cv[j] < 0:
                cv, rv = -cv, -rv
            for g in groups:
                if np.abs(g.cv - cv).max() < 1e-5:
                    g.terms.append((b, rv))
                    break
            else:
                g = _Group(cv)
                g.terms.append((b, rv))
                groups.append(g)
    return groups


def _pick_scales(groups):
    odd_g = int(np.argmax([abs(g.odd_taps[1]) for g in groups]))
    groups[odd_g].scale = groups[odd_g].odd_taps[1]
    ev_cand = [(i, abs(g.even_taps[1])) for i, g in enumerate(groups) if i != odd_g]
    extra_even_scale = None
    if ev_cand:
        ev_g = max(ev_cand, key=lambda t: t[1])[0]
        groups[ev_g].scale = groups[ev_g].even_taps[1]
    else:
        ev_g = odd_g
        extra_even_scale = groups[ev_g].even_taps[1] / groups[ev_g].scale
    for i, g in enumerate(groups):
        if i not in (odd_g, ev_g):
            g.scale = g.odd_taps[1] if abs(g.odd_taps[1]) > abs(g.even_taps[1]) \
                else g.even_taps[1]

    even_chain, odd_chain = [], []
    for i, g in enumerate(groups):
        a, b = g.even_taps
        even_chain.append((i, -1, a / g.scale))
        if i != ev_g or extra_even_scale is not None:
            even_chain.append((i, 0, b / g.scale))
    if extra_even_scale is None:
        even_chain.append((ev_g, 0, 1.0))
    for i, g in enumerate(groups):
        a, b, c = g.odd_taps
        odd_chain.append((i, -1, a / g.scale))
        odd_chain.append((i, +1, c / g.scale))
        if i != odd_g:
            odd_chain.append((i, 0, b / g.scale))
    odd_chain.append((odd_g, 0, 1.0))
    even_chain = [t for t in even_chain if abs(t[2]) > 1e-12]
    odd_chain = [t for t in odd_chain if abs(t[2]) > 1e-12]
    return even_chain, odd_chain, extra_even_scale


def _fixups(groups):
    fix = {0: [], 1: [], OUT - 1: []}
    for i, g in enumerate(groups):
        c0, c1 = g.cv
        s = g.scale
        fix[0].append((i, 0, c1 / s))
        fix[1].append((i, 0, (c0 + 0.75 * c1) / s))
        fix[1].append((i, 1, 0.25 * c1 / s))
        fix[OUT - 1].append((i, H - 2, 0.25 * c0 / s))
        fix[OUT - 1].append((i, H - 1, (0.75 * c0 + c1) / s))
    for col in fix:
        fix[col] = [t for t in fix[col] if abs(t[2]) > 1e-12]
    return fix


def _build_slabs(groups):
    out = []
    blocks = []
    s = 0
    while s < OUT:
        m = min(BM, OUT - s)
        blocks.append((s, m))
        s += m
    for g in groups:
        Rs = [_band_matrix(rv, H) for _, rv in g.terms]
        entries = []
        for (s, m) in blocks:
            windows, pieces = [], []
            for (band, _), R in zip(g.terms, Rs):
                sub = R[s:s + m]
                cols = np.nonzero(np.any(sub != 0.0, axis=0))[0]
                k0, k1 = int(cols.min()), int(cols.max()) + 1
                windows.append((band, k0, k1 - k0))
                pieces.append(sub[:, k0:k1].T)
            slab = np.concatenate(pieces, axis=0).astype(np.float32)
            assert slab.shape[0] <= 128, f"K={slab.shape[0]} > 128"
            entries.append((s, m, windows, slab))
        out.append(entries)
    return out


def _build_program(filters):
    groups = _decompose(filters)
    even_chain, odd_chain, extra_even_scale = _pick_scales(groups)
    fix = _fixups(groups)
    slabs = _build_slabs(groups)
    G = len(groups)
    NB = len(slabs[0])

    all_slabs = []
    slab_idx = {}
    for gi in range(G):
        for bi, (s, m, w, slab) in enumerate(slabs[gi]):
            pad = np.zeros((128, BM), dtype=np.float32)
            pad[:slab.shape[0], :slab.shape[1]] = slab
            slab_idx[(gi, bi)] = len(all_slabs)
            all_slabs.append(pad)
    slab_np = np.stack(all_slabs)  # [NS, 128, BM]
    NS = slab_np.shape[0]

    nc = bacc.Bacc("TRN2", target_bir_lowering=False, debug=False,
                   num_devices=N_CORES)
    x = nc.dram_tensor("x", [SPC, 4 * CH, H, H], F32, kind="ExternalInput").ap()
    sl = nc.dram_tensor("slabs", [NS, 128, BM], F32, kind="ExternalInput").ap()
    y = nc.dram_tensor("y", [SPC, CH, OUT, OUT], F32, kind="ExternalOutput").ap()

    with tile.TileContext(nc) as tc:
        with (
            tc.tile_pool(name="const", bufs=1) as cpool,
            tc.tile_pool(name="rhs", bufs=4) as rpool,
            tc.tile_pool(name="psum", bufs=4, space="PSUM") as ppool,
            tc.tile_pool(name="ypool", bufs=3) as ypool,
            tc.tile_pool(name="opool", bufs=2) as opool,
            tc.tile_pool(name="tpool", bufs=2) as tpool,
            tc.tile_pool(name="fpool", bufs=4) as fpool,
        ):
            slab_t = cpool.tile([128, NS * BM], F32)
            for i in range(NS):
                nc.sync.dma_start(slab_t[:, bass.ts(i, BM)], sl[i])

            for sc in range(SPC * CH):
                sample, ch = divmod(sc, CH)
                Y = [ypool.tile([120, 9, 516], F32, tag="Y", name=f"Y{sc}_{g_}")
                     for g_ in range(G)]
                for gi in range(G):
                    scale = float(groups[gi].scale)
                    pt = None
                    for bi, (s, m, windows, slab) in enumerate(slabs[gi]):
                        K = slab.shape[0]
                        rhs = rpool.tile([128, H], F32)
                        koff = 0
                        for band, k0, kw in windows:
                            nc.sync.dma_start(
                                rhs[koff:koff + kw, :],
                                x[sample, band * CH + ch, k0:k0 + kw, :])
                            koff += kw
                        if bi % 2 == 0:
                            pt = ppool.tile([120, 2, 512], F32)
                        si = slab_idx[(gi, bi)]
                        nc.tensor.matmul(
                            pt[:m, bi % 2, :],
                            slab_t[:K, si * BM:si * BM + m],
                            rhs[:K, :], start=True, stop=True)
                        if bi % 2 == 1:
                            nc.scalar.mul(Y[gi][:, bi - 1:bi + 1, 2:514],
                                          pt[:, 0:2, :], scale)
                        elif bi == NB - 1:
                            nc.scalar.mul(Y[gi][:m, bi:bi + 1, 2:514],
                                          pt[:m, 0:1, :], scale)

                def tap(gi_, t_):
                    return Y[gi_][:, :, 2 + t_:514 + t_]

                O = opool.tile([120, 9, OUT], F32)

                def run_chain(chain, out_view):
                    g0, t0, c0 = chain[0]
                    prev, prev_c = tap(g0, t0), c0
                    for idx, (gi_, ti_, ci_) in enumerate(chain[1:]):
                        last = idx == len(chain) - 2
                        dst = out_view if last else tpool.tile(
                            [120, 9, 512], F32, tag="tmp", name=f"t{sc}_{idx}")
                        nc.vector.scalar_tensor_tensor(
                            dst, prev, float(prev_c / ci_), tap(gi_, ti_),
                            mybir.AluOpType.mult, mybir.AluOpType.add)
                        prev, prev_c = dst, ci_
                    return prev

                ev = run_chain(even_chain, O[:, :, 0:OUT:2])
                if extra_even_scale is not None:
                    nc.vector.tensor_scalar_mul(ev, ev, float(extra_even_scale))
                run_chain(odd_chain, O[:, :, 1:OUT:2])

                for col, lst in fix.items():
                    acc = None
                    for i, (gi_, ycol, cf) in enumerate(lst):
                        tv = Y[gi_][:, :, 2 + ycol:3 + ycol]
                        last = i == len(lst) - 1
                        dst = O[:, :, col:col + 1] if last else fpool.tile(
                            [120, 9, 1], F32, tag="fx", name=f"f{sc}_{col}_{i}")
                        if acc is None:
                            if last:
                                nc.vector.tensor_scalar_mul(dst, tv, float(cf))
                            else:
                                nc.vector.tensor_scalar_mul(dst, tv, float(cf))
                            acc = dst
                        else:
                            nc.vector.scalar_tensor_tensor(
                                dst, tv, float(cf), acc,
                                mybir.AluOpType.mult, mybir.AluOpType.add)
                            acc = dst

                nc.sync.dma_start(
                    y[sample, ch, 0:960, :].rearrange("(g p) w -> p g w", p=120),
                    O[:, 0:8, :])
                nc.sync.dma_start(y[sample, ch, 960:OUT, :], O[0:64, 8, :])

    nc.compile()
    return nc, slab_np


_CACHE = {}


def kernel(x, fll, flh, fhl, fhh):
    x = np.ascontiguousarray(np.asarray(x, dtype=np.float32))
    filters = [np.asarray(f, dtype=np.float32) for f in (fll, flh, fhl, fhh)]
    key = b"".join(f.tobytes() for f in filters)
    if key not in _CACHE:
        _CACHE[key] = _build_program(filters)
    nc, slab_np = _CACHE[key]
    in_maps = [{"x": x[c * SPC:(c + 1) * SPC], "slabs": slab_np}
               for c in range(N_CORES)]
    res = run_bass_kernel_spmd(nc, in_maps, core_ids=list(range(N_CORES)))
    return np.concatenate([res.results[c]["y"] for c in range(N_CORES)], axis=0)


# revision 3
# speedup vs baseline: 276.3782x; 276.3782x over previous
"""Trainium2 Bass kernel for nn_InverseHaarTransform.

out = sum_b upfirdn(band_b, f_b) for 4 bands; reformulated per (sample,ch) as
out = sum_g R_g @ u_pair_g @ C_g^T with banded 1024x512 operators derived from
rank-1 (SVD) factors of each 2x2 filter.

Implementation per NeuronCore (2 samples x 3 channels = 6 instances):
  - H-pass on TensorE: fp32 matmuls, stationary slab [K<=128, m=120] packs both
    bands of a column-factor group; rhs packs the two bands' input row windows.
  - W-pass on VectorE: scalar_tensor_tensor chains over column-shifted views of
    Y, writing even/odd output columns directly with stride-2 APs. Per-group
    pre-scales (folded into the ACT PSUM->SBUF copy) make chains end at coef 1.
  - Boundary output columns 0, 1, 1023 get exact small fixup ops.
Batch dim sharded 2-per-core across 8 cores; filters replicated.
"""
import sys
sys.path.insert(0, "/opt/trn_rl_repo")
import numpy as np
import concourse.bass as bass
import concourse.bacc as bacc
import concourse.tile as tile
import concourse.mybir as mybir
from concourse.bass_utils import run_bass_kernel_spmd

F32 = mybir.dt.float32
H = 512
OUT = 1024
BM = 120
N_CORES = 8
SPC = 2   # samples per core
CH = 3    # output channels


def _up_matrix(n):
    A = np.zeros((2 * n, n))
    A[0, 0] = 1.0
    for k in range(1, n):
        A[2 * k, k - 1] = 0.25
        A[2 * k, k] = 0.75
    for k in range(0, n - 1):
        A[2 * k + 1, k] = 0.75
        A[2 * k + 1, k + 1] = 0.25
    A[2 * n - 1, n - 1] = 1.0
    return A


def _band_matrix(rv, n):
    A = _up_matrix(n)
    S = np.zeros_like(A)
    S[1:] = A[:-1]
    return rv[0] * S + rv[1] * A


class _Group:
    def __init__(self, cv):
        self.cv = cv
        self.terms = []
        self.scale = 1.0

    @property
    def even_taps(self):
        c0, c1 = self.cv
        return (0.75 * c0 + 0.25 * c1, 0.25 * c0 + 0.75 * c1)

    @property
    def odd_taps(self):
        c0, c1 = self.cv
        return (0.25 * c0, 0.75 * (c0 + c1), 0.25 * c1)


def _decompose(filters):
    groups = []
    for b, f in enumerate(filters):
        U, s, Vt = np.linalg.svd(np.asarray(f, dtype=np.float64))
        for t in range(2):
            if s[t] <= 1e-7 * max(s[0], 1e-30):
                continue
            rv = U[:, t] * s[t]
            cv = Vt[t, :]
            j = int(np.argmax(np.abs(cv)))
            if cv[j] < 0:
                cv, rv = -cv, -rv
            for g in groups:
                if np.abs(g.cv - cv).max() < 1e-5:
                    g.terms.append((b, rv))
                    break
            else:
                g = _Group(cv)
                g.terms.append((b, rv))
                groups.append(g)
    return groups


def _pick_scales(groups):
    odd_g = int(np.argmax([abs(g.odd_taps[1]) for g in groups]))
    groups[odd_g].scale = groups[odd_g].odd_taps[1]
    ev_cand = [(i, abs(g.even_taps[1])) for i, g in enumerate(groups) if i != odd_g]
    extra_even_scale = None
    if ev_cand:
        ev_g = max(ev_cand, key=lambda t: t[1])[0]
        groups[ev_g].scale = groups[ev_g].even_taps[1]
    else:
        ev_g = odd_g
        extra_even_scale = groups[ev_g].even_taps[1] / groups[ev_g].scale
    for i, g in enumerate(groups):
        if i not in (odd_g, ev_g):
            g.scale = g.odd_taps[1] if abs(g.odd_taps[1]) > abs(g.even_taps[1]) \
                else g.even_taps[1]

    even_chain, odd_chain = [], []
    for i, g in enumerate(groups):
        a, b = g.even_taps
        even_chain.append((i, -1, a / g.scale))
        if i != ev_g or extra_even_scale is not None:
            even_chain.append((i, 0, b / g.scale))
    if extra_even_scale is None:
        even_chain.append((ev_g, 0, 1.0))
    for i, g in enumerate(groups):
        a, b, c = g.odd_taps
        odd_chain.append((i, -1, a / g.scale))
        odd_chain.append((i, +1, c / g.scale))
        if i != odd_g:
            odd_chain.append((i, 0, b / g.scale))
    odd_chain.append((odd_g, 0, 1.0))
    even_chain = [t for t in even_chain if abs(t[2]) > 1e-12]
    odd_chain = [t for t in odd_chain if abs(t[2]) > 1e-12]
    return even_chain, odd_chain, extra_even_scale


def _fixups(groups):
    fix = {0: [], 1: [], OUT - 1: []}
    for i, g in enumerate(groups):
        c0, c1 = g.cv
        s = g.scale
        fix[0].append((i, 0, c1 / s))
        fix[1].append((i, 0, (c0 + 0.75 * c1) / s))
        fix[1].append((i, 1, 0.25 * c1 / s))
        fix[OUT - 1].append((i, H - 2, 0.25 * c0 / s))
        fix[OUT - 1].append((i, H - 1, (0.75 * c0 + c1) / s))
    for col in fix:
        fix[col] = [t for t in fix[col] if abs(t[2]) > 1e-12]
    return fix


def _build_slabs(groups):
    out = []
    blocks = []
    s = 0
    while s < OUT:
        m = min(BM, OUT - s)
        blocks.append((s, m))
        s += m
    for g in groups:
        Rs = [_band_matrix(rv, H) for _, rv in g.terms]
        entries = []
        for (s, m) in blocks:
            windows, pieces = [], []
            for (band, _), R in zip(g.terms, Rs):
                sub = R[s:s + m]
                cols = np.nonzero(np.any(sub != 0.0, axis=0))[0]
                k0, k1 = int(cols.min()), int(cols.max()) + 1
                windows.append((band, k0, k1 - k0))
                pieces.append(sub[:, k0:k1].T)
            slab = np.concatenate(pieces, axis=0).astype(np.float32)
            assert slab.shape[0] <= 128, f"K={slab.shape[0]} > 128"
            entries.append((s, m, windows, slab))
        out.append(entries)
    return out


def _build_program(filters):
    groups = _decompose(filters)
    even_chain, odd_chain, extra_even_scale = _pick_scales(groups)
    fix = _fixups(groups)
    slabs = _build_slabs(groups)
    G = len(groups)
    NB = len(slabs[0])

    all_slabs = []
    slab_idx = {}
    for gi in range(G):
        for bi, (s, m, w, slab) in enumerate(slabs[gi]):
            pad = np.zeros((128, BM), dtype=np.float32)
            pad[:slab.shape[0], :slab.shape[1]] = slab
            slab_idx[(gi, bi)] = len(all_slabs)
            all_slabs.append(pad)
    slab_np = np.stack(all_slabs)  # [NS, 128, BM]
    NS = slab_np.shape[0]

    nc = bacc.Bacc("TRN2", target_bir_lowering=False, debug=False,
                   num_devices=N_CORES)
    x = nc.dram_tensor("x", [SPC, 4 * CH, H, H], F32, kind="ExternalInput").ap()
    sl = nc.dram_tensor("slabs", [NS, 128, BM], F32, kind="ExternalInput").ap()
    y = nc.dram_tensor("y", [SPC, CH, OUT, OUT], F32, kind="ExternalOutput").ap()

    with tile.TileContext(nc) as tc:
        with (
            tc.tile_pool(name="const", bufs=1) as cpool,
            tc.tile_pool(name="rhs", bufs=6) as rpool,
            tc.tile_pool(name="psum", bufs=4, space="PSUM") as ppool,
            tc.tile_pool(name="ypool", bufs=4) as ypool,
            tc.tile_pool(name="opool", bufs=2) as opool,
            tc.tile_pool(name="tpool", bufs=2) as tpool,
            tc.tile_pool(name="fpool", bufs=4) as fpool,
        ):
            slab_t = cpool.tile([128, NS * BM], F32)
            for i in range(NS):
                nc.sync.dma_start(slab_t[:, bass.ts(i, BM)], sl[i])

            for sc in range(SPC * CH):
                sample, ch = divmod(sc, CH)
                Y = [ypool.tile([120, 9, 516], F32, tag="Y", name=f"Y{sc}_{g_}")
                     for g_ in range(G)]
                for gi in range(G):
                    scale = float(groups[gi].scale)
                    pt = None
                    for bi, (s, m, windows, slab) in enumerate(slabs[gi]):
                        K = slab.shape[0]
                        rhs = rpool.tile([128, H], F32)
                        koff = 0
                        for band, k0, kw in windows:
                            nc.sync.dma_start(
                                rhs[koff:koff + kw, :],
                                x[sample, band * CH + ch, k0:k0 + kw, :])
                            koff += kw
                        if bi % 2 == 0:
                            pt = ppool.tile([120, 2, 512], F32)
                        si = slab_idx[(gi, bi)]
                        nc.tensor.matmul(
                            pt[:m, bi % 2, :],
                            slab_t[:K, si * BM:si * BM + m],
                            rhs[:K, :], start=True, stop=True)
                        if bi % 2 == 1:
                            nc.scalar.mul(Y[gi][:, bi - 1:bi + 1, 2:514],
                                          pt[:, 0:2, :], scale)
                        elif bi == NB - 1:
                            nc.scalar.mul(Y[gi][:m, bi:bi + 1, 2:514],
                                          pt[:m, 0:1, :], scale)

                def tap(gi_, t_):
                    return Y[gi_][:, :, 2 + t_:514 + t_]

                O = opool.tile([120, 9, OUT], F32)

                def run_chain(chain, out_view):
                    g0, t0, c0 = chain[0]
                    prev, prev_c = tap(g0, t0), c0
                    for idx, (gi_, ti_, ci_) in enumerate(chain[1:]):
                        last = idx == len(chain) - 2
                        dst = out_view if last else tpool.tile(
                            [120, 9, 512], F32, tag="tmp", name=f"t{sc}_{idx}")
                        nc.vector.scalar_tensor_tensor(
                            dst, prev, float(prev_c / ci_), tap(gi_, ti_),
                            mybir.AluOpType.mult, mybir.AluOpType.add)
                        prev, prev_c = dst, ci_
                    return prev

                ev = run_chain(even_chain, O[:, :, 0:OUT:2])
                if extra_even_scale is not None:
                    nc.vector.tensor_scalar_mul(ev, ev, float(extra_even_scale))
                run_chain(odd_chain, O[:, :, 1:OUT:2])

                for col, lst in fix.items():
                    acc = None
                    for i, (gi_, ycol, cf) in enumerate(lst):
                        tv = Y[gi_][:, :, 2 + ycol:3 + ycol]
                        last = i == len(lst) - 1
                        dst = O[:, :, col:col + 1] if last else fpool.tile(
                            [120, 9, 1], F32, tag="fx", name=f"f{sc}_{col}_{i}")
                        if acc is None:
                            if last:
                                nc.vector.tensor_scalar_mul(dst, tv, float(cf))
                            else:
                                nc.vector.tensor_scalar_mul(dst, tv, float(cf))
                            acc = dst
                        else:
                            nc.vector.scalar_tensor_tensor(
                                dst, tv, float(cf), acc,
                                mybir.AluOpType.mult, mybir.AluOpType.add)
                            acc = dst

                nc.sync.dma_start(
                    y[sample, ch, 0:960, :].rearrange("(g p) w -> p g w", p=120),
                    O[:, 0:8, :])
                nc.sync.dma_start(y[sample, ch, 960:OUT, :], O[0:64, 8, :])

    nc.compile()
    return nc, slab_np


_CACHE = {}


def kernel(x, fll, flh, fhl, fhh):
    x = np.ascontiguousarray(np.asarray(x, dtype=np.float32))
    filters = [np.asarray(f, dtype=np.float32) for f in (fll, flh, fhl, fhh)]
    key = b"".join(f.tobytes() for f in filters)
    if key not in _CACHE:
        _CACHE[key] = _build_program(filters)
    nc, slab_np = _CACHE[key]
    in_maps = [{"x": x[c * SPC:(c + 1) * SPC], "slabs": slab_np}
               for c in range(N_CORES)]
    res = run_bass_kernel_spmd(nc, in_maps, core_ids=list(range(N_CORES)))
    return np.concatenate([res.results[c]["y"] for c in range(N_CORES)], axis=0)
